# revision 4
# baseline (speedup 1.0000x reference)
"""CRF log-partition kernel V3 for Trainium2: 3-chunk decomposition.

Splits the 512-step product logZ_b = log q^T M_{L-1}..M_0 alpha_0 into three
chunks [0,171), [171,342), [342,512). Sequential depth drops to 171 steps.

Per-element composition (host, f64):
  L <= 342:  logZ = log( a · v1(L) ) + folds                      (exact)
  L  > 342:  logZ = log( v2(L)·p1 ) - log( e·p1 ) + log( r1·a ) + folds
where
  a     = M0 alpha0               (fwd chain, chunk 0, true start)
  p1    = M1 e                    (fwd probe chain, chunk 1, start e)
  v1(L) = (M1^{part to L})^T q    (bwd repacked chain, chunk 1)
  r1    = M1^T e                  (same chunk-1 bwd chain when started at e)
  v2(L) = (M2^{part to L})^T q    (bwd repacked chain, chunk 2)
The L>342 branch replaces M1 by its rank-1 compression p1 (e^T M1)/(e^T M1 e);
a product of 171 positive matrices contracts to rank-1 far below bf16 noise,
so this is numerically exact in practice.

bwd chains use per-element time-repacking: slot j holds the emissions of
step top_b-1-j (top_b = min(L_b, chunk_end)); the junction slot
j* = top_b - chunk_start carries multiplier 1 and later slots 0, so the
state history at j* is the exact per-element partial product. Masks never
touch the device.

Device: TWO tile-chains of 171 steps, state [112,128] bf16 (rows 0-47 a fwd
chain over all 128 elements, rows 64-111 a bwd chain), one shared stationary
W[112,112] = blockdiag(E^T, 0, E) loaded into the PE array exactly once, per
step ONE matmul + ONE VectorE multiply per chain. Two chains pipeline the
TensorE->VectorE->TensorE latency; the [112,128] multiply amortizes the
VectorE PSUM-access penalty over all 256 element-chains.
"""

import os
import sys

import numpy as np

for _p in ("/opt/trn_rl_repo",):
    if _p not in sys.path and os.path.isdir(_p):
        sys.path.insert(0, _p)

import ml_dtypes  # noqa: E402

import concourse.bass as bass  # noqa: E402
import concourse.bacc as bacc  # noqa: E402
import concourse.mybir as mybir  # noqa: E402
from concourse import tile  # noqa: E402
from concourse.bass_utils import run_bass_kernel_spmd  # noqa: E402

BF16 = ml_dtypes.bfloat16

B, S, T = 1024, 512, 48
NCORES = 8
BSH = B // NCORES          # 128 elements per core
GB = BSH                   # elements per tile-chain (full core)
B1, B2 = 171, 342          # chunk boundaries
HS = 171                   # device steps per tile-chain
NSLOT = HS + 1             # 172 history slots
NTC = 2                    # tile-chains per core
START_IDX, STOP_IDX = 45, 46
FOLD_C = 4.8
P = 112
# input-chunk step boundaries: small first chunk so step 0 starts early
IN_BOUNDS = [8, 32, 64, 96, 128, 160, HS]


def build_nc():
    f32 = mybir.dt.float32
    bf = mybir.dt.bfloat16
    nc = bacc.Bacc("TRN2", target_bir_lowering=False, debug=False)
    w_d = nc.declare_dram_parameter("W", [P, P], bf, isOutput=False)
    init_d = nc.declare_dram_parameter("init", [P, NTC * GB], bf, isOutput=False)
    mult_d = [
        nc.declare_dram_parameter(f"mult{g}", [P, HS * GB], bf, isOutput=False)
        for g in range(NTC)
    ]
    hist_d = [
        nc.declare_dram_parameter(f"hist{g}", [T, NSLOT * GB], bf, isOutput=True)
        for g in range(NTC)
    ]
    fin_d = nc.declare_dram_parameter("fin", [T, NTC * GB], bf, isOutput=True)

    with tile.TileContext(nc) as tc:
        with (
            tc.tile_pool(name="const", bufs=1) as constp,
            tc.tile_pool(name="arena", bufs=1) as arenap,
            tc.tile_pool(name="ring", bufs=1) as ringp,
            tc.tile_pool(name="ps", bufs=2, space="PSUM") as psp,
        ):
            w_t = constp.tile([P, P], bf, tag="W")
            nc.sync.dma_start(w_t[:], w_d[:])
            rings, arenas = [], []
            for g in range(NTC):
                ring = ringp.tile([P, NSLOT * GB], bf, tag=f"ring{g}")
                nc.sync.dma_start(ring[:, 0:GB], init_d[:, g * GB : (g + 1) * GB])
                rings.append(ring)
                arena = arenap.tile([P, HS * GB], bf, tag=f"mult{g}")
                arenas.append(arena)
            prev = 0
            for bnd in IN_BOUNDS:
                sl = slice(prev * GB, bnd * GB)
                prev = bnd
                for g in range(NTC):
                    nc.sync.dma_start(arenas[g][:, sl], mult_d[g][:, sl])

            out_bounds = sorted(
                set(range(32, HS - 32 + 1, 32)) | set(range(HS - 32, HS + 1, 8))
            )
            prev_b = 0
            for k in range(HS):
                for g in range(NTC):
                    ps_t = psp.tile([P, GB], f32, tag=f"ps{g}")
                    nc.tensor.matmul(
                        ps_t[:],
                        w_t[:],
                        rings[g][:, k * GB : (k + 1) * GB],
                        start=True,
                        stop=True,
                    )
                    nc.vector.tensor_mul(
                        rings[g][:, (k + 1) * GB : (k + 2) * GB],
                        ps_t[:],
                        arenas[g][:, k * GB : (k + 1) * GB],
                    )
                if (k + 1) in out_bounds:
                    sl = slice(prev_b * GB, (k + 1) * GB)
                    prev_b = k + 1
                    for g in range(NTC):
                        nc.sync.dma_start(
                            hist_d[g][:, sl], rings[g][64 : 64 + T, sl]
                        )
            for g in range(NTC):
                sl = slice(HS * GB, (HS + 1) * GB)
                nc.sync.dma_start(hist_d[g][:, sl], rings[g][64 : 64 + T, sl])
                nc.sync.dma_start(
                    fin_d[:, g * GB : (g + 1) * GB], rings[g][0:T, sl]
                )
    _prune_ldweights(nc)
    nc.compile()
    return nc


def _prune_ldweights(nc):
    """All matmuls share one stationary: keep only the first InstLdweights
    (it carries the weight-DMA wait), mark matmuls non-self-loading."""
    first = True
    for blk in nc.m.functions[0].blocks:
        keep = []
        for inst in blk.instructions:
            tn = type(inst).__name__
            if tn == "InstMatmult":
                inst.ldweights = False
                keep.append(inst)
            elif tn == "InstLdweights":
                si = inst.sync_info
                has_sync = si is not None and (
                    len(si.on_wait) > 0 or len(si.on_update) > 0
                )
                if first or has_sync:
                    keep.append(inst)
                    first = False
            else:
                keep.append(inst)
        if len(keep) != len(blk.instructions):
            blk.instructions[:] = keep


def _bwd_pack(f64, lengths, cstart, cend):
    """Repacked bwd multiplier slots for chunk [cstart, cend).
    Returns (slot0 mult [B,T], device mults [B,HS,T], fold [B], jstar [B])."""
    top = np.minimum(lengths, cend)
    jstar = top - cstart                                 # [B] in [0, HS]
    slot = np.arange(NSLOT)[None, :]
    step_idx = top[:, None] - 1 - slot
    real = slot < jstar[:, None]
    step_clip = np.clip(step_idx, 0, S - 1)
    f_rep = f64[np.arange(B)[:, None], step_clip, :]
    g = (f_rep.mean(axis=-1) + FOLD_C) * real
    ef = np.exp(f_rep - g[..., None])
    ef = np.where(real[..., None], ef, 0.0)
    ef[np.arange(B), jstar, :] = 1.0
    return ef[:, 0, :], ef[:, 1:, :], g.sum(axis=1), jstar


def host_prep(feats, transitions, lengths):
    A = transitions.astype(np.float64)
    E = np.exp(A)
    q = np.exp(A[STOP_IDX, :])
    e_probe = np.ones(T)

    W = np.zeros((P, P), np.float64)
    W[0:T, 0:T] = E.T                 # fwd block: psum rows 0-47 = E @ state
    W[64 : 64 + T, 64 : 64 + T] = E   # bwd block: psum rows 64-111 = E^T @ w
    W_bf = W.astype(BF16)

    f64 = feats.astype(np.float64)
    gf = f64.mean(axis=-1) + FOLD_C                       # [B,S]
    ef_all = np.exp(f64 - gf[..., None])                  # [B,S,T]
    fold_f0 = gf[:, 0:B1].sum(axis=1)

    w0m_1, m_1, fold_b1, js1 = _bwd_pack(f64, lengths, B1, B2)
    w0m_2, m_2, fold_b2, js2 = _bwd_pack(f64, lengths, B2, S)
    ctop = lengths > B2                                   # c* == 2

    start1 = np.where(ctop[:, None], e_probe[None, :], q[None, :])
    w0_1 = w0m_1 * start1
    w0_2 = w0m_2 * q[None, :]

    inits, mults = [], []
    for c in range(NCORES):
        esl = slice(c * BSH, (c + 1) * BSH)
        init = np.zeros((P, NTC * GB), np.float64)
        ms = []
        for g in range(NTC):
            isl = slice(g * GB, (g + 1) * GB)
            if g == 0:
                init[START_IDX, isl] = 1.0                # alpha_0
                init[64 : 64 + T, isl] = w0_1[esl].T
                fm = ef_all[esl, 0:B1, :]                 # [GB,HS,T]
                bm = m_1[esl]
            else:
                init[0:T, isl] = e_probe[:, None]         # probe start
                init[64 : 64 + T, isl] = w0_2[esl].T
                fm = ef_all[esl, B1:B2, :]
                bm = m_2[esl]
            m = np.zeros((P, HS, GB), np.float64)
            m[0:T] = fm.transpose(2, 1, 0)
            m[64 : 64 + T] = bm.transpose(2, 1, 0)
            ms.append(np.ascontiguousarray(m.reshape(P, HS * GB)).astype(BF16))
        inits.append(init.astype(BF16))
        mults.append(ms)
    folds = dict(f0=fold_f0, b1=fold_b1, b2=fold_b2)
    return W_bf, inits, mults, folds, js1, js2, ctop, e_probe


def host_finish(hists, fins, folds, js1, js2, ctop, e_probe):
    """hists: [NCORES][NTC] of [T, NSLOT*GB]; fins: [NCORES][T, NTC*GB]."""
    out = np.empty((B,), np.float32)
    e_idx = np.arange(GB)
    for c in range(NCORES):
        bsl = slice(c * BSH, (c + 1) * BSH)
        fin = fins[c].astype(np.float64)
        a = fin[:, 0:GB]                                  # [T,GB] chunk0 fwd
        p1 = fin[:, GB : 2 * GB]                          # [T,GB] chunk1 probe
        h1 = hists[c][0].astype(np.float64).reshape(T, NSLOT, GB)
        h2 = hists[c][1].astype(np.float64).reshape(T, NSLOT, GB)
        v1 = h1[:, js1[bsl], e_idx]                       # [T,GB]
        v2 = h2[:, js2[bsl], e_idx]
        r1 = h1[:, HS, :]                                 # [T,GB] full chunk1
        low = np.log((a * v1).sum(axis=0)) + folds["f0"][bsl] + folds["b1"][bsl]
        high = (
            np.log((v2 * p1).sum(axis=0))
            - np.log((e_probe[:, None] * p1).sum(axis=0))
            + np.log((r1 * a).sum(axis=0))
            + folds["f0"][bsl]
            + folds["b1"][bsl]
            + folds["b2"][bsl]
        )
        out[bsl] = np.where(ctop[bsl], high, low).astype(np.float32)
    return out


def _run(feats, transitions, masks, trace=False):
    feats = np.asarray(feats)
    transitions = np.asarray(transitions)
    masks = np.asarray(masks)
    lengths = masks.sum(axis=1).astype(np.int64)

    W_bf, inits, mults, folds, js1, js2, ctop, e_probe = host_prep(
        feats, transitions, lengths
    )
    in_maps = [
        {
            "W": W_bf,
            "init": inits[c],
            **{f"mult{g}": mults[c][g] for g in range(NTC)},
        }
        for c in range(NCORES)
    ]
    nc = build_nc()
    bres = run_bass_kernel_spmd(
        nc, in_maps, core_ids=list(range(NCORES)), trace=trace
    )
    hists = [[r[f"hist{g}"] for g in range(NTC)] for r in bres.results]
    fins = [r["fin"] for r in bres.results]
    return host_finish(hists, fins, folds, js1, js2, ctop, e_probe), bres


def kernel(feats, transitions, masks):
    out, _ = _run(feats, transitions, masks, trace=False)
    return out


# revision 5
# speedup vs baseline: 1.0047x; 1.0047x over previous
"""CRF log-partition kernel V3 for Trainium2: 3-chunk decomposition.

Splits the 512-step product logZ_b = log q^T M_{L-1}..M_0 alpha_0 into three
chunks [0,171), [171,342), [342,512). Sequential depth drops to 171 steps.

Per-element composition (host, f64):
  L <= 342:  logZ = log( a · v1(L) ) + folds                      (exact)
  L  > 342:  logZ = log( v2(L)·p1 ) - log( e·p1 ) + log( r1·a ) + folds
where
  a     = M0 alpha0               (fwd chain, chunk 0, true start)
  p1    = M1 e                    (fwd probe chain, chunk 1, start e)
  v1(L) = (M1^{part to L})^T q    (bwd repacked chain, chunk 1)
  r1    = M1^T e                  (same chunk-1 bwd chain when started at e)
  v2(L) = (M2^{part to L})^T q    (bwd repacked chain, chunk 2)
The L>342 branch replaces M1 by its rank-1 compression p1 (e^T M1)/(e^T M1 e);
a product of 171 positive matrices contracts to rank-1 far below bf16 noise,
so this is numerically exact in practice.

bwd chains use per-element time-repacking: slot j holds the emissions of
step top_b-1-j (top_b = min(L_b, chunk_end)); the junction slot
j* = top_b - chunk_start carries multiplier 1 and later slots 0, so the
state history at j* is the exact per-element partial product. Masks never
touch the device.

Device: TWO tile-chains of 171 steps, state [112,128] bf16 (rows 0-47 a fwd
chain over all 128 elements, rows 64-111 a bwd chain), one shared stationary
W[112,112] = blockdiag(E^T, 0, E) loaded into the PE array exactly once, per
step ONE matmul + ONE VectorE multiply per chain. Two chains pipeline the
TensorE->VectorE->TensorE latency; the [112,128] multiply amortizes the
VectorE PSUM-access penalty over all 256 element-chains.
"""

import os
import sys

import numpy as np

for _p in ("/opt/trn_rl_repo",):
    if _p not in sys.path and os.path.isdir(_p):
        sys.path.insert(0, _p)

import ml_dtypes  # noqa: E402

import concourse.bass as bass  # noqa: E402
import concourse.bacc as bacc  # noqa: E402
import concourse.mybir as mybir  # noqa: E402
from concourse import tile  # noqa: E402
from concourse.bass_utils import run_bass_kernel_spmd  # noqa: E402

BF16 = ml_dtypes.bfloat16

B, S, T = 1024, 512, 48
NCORES = 8
BSH = B // NCORES          # 128 elements per core
GB = BSH                   # elements per tile-chain (full core)
B1, B2 = 171, 342          # chunk boundaries
HS = 171                   # device steps per tile-chain
NSLOT = HS + 1             # 172 history slots
NTC = 2                    # tile-chains per core
START_IDX, STOP_IDX = 45, 46
FOLD_C = 4.8
P = 112
# input-chunk step boundaries: small first chunk so step 0 starts early
IN_BOUNDS = [8, 32, 64, 96, 128, 160, HS]


def build_nc():
    f32 = mybir.dt.float32
    bf = mybir.dt.bfloat16
    nc = bacc.Bacc("TRN2", target_bir_lowering=False, debug=False)
    w_d = nc.declare_dram_parameter("W", [P, P], bf, isOutput=False)
    init_d = nc.declare_dram_parameter("init", [P, NTC * GB], bf, isOutput=False)
    mult_d = [
        nc.declare_dram_parameter(f"mult{g}", [P, HS * GB], bf, isOutput=False)
        for g in range(NTC)
    ]
    hist_d = [
        nc.declare_dram_parameter(f"hist{g}", [T, NSLOT * GB], bf, isOutput=True)
        for g in range(NTC)
    ]
    fin_d = nc.declare_dram_parameter("fin", [T, NTC * GB], bf, isOutput=True)

    with tile.TileContext(nc) as tc:
        with (
            tc.tile_pool(name="const", bufs=1) as constp,
            tc.tile_pool(name="arena", bufs=1) as arenap,
            tc.tile_pool(name="ring", bufs=1) as ringp,
            tc.tile_pool(name="ps", bufs=2, space="PSUM") as psp,
        ):
            w_t = constp.tile([P, P], bf, tag="W")
            nc.sync.dma_start(w_t[:], w_d[:])
            rings, arenas = [], []
            for g in range(NTC):
                ring = ringp.tile([P, NSLOT * GB], bf, tag=f"ring{g}")
                nc.sync.dma_start(ring[:, 0:GB], init_d[:, g * GB : (g + 1) * GB])
                rings.append(ring)
                arena = arenap.tile([P, HS * GB], bf, tag=f"mult{g}")
                arenas.append(arena)
            prev = 0
            for bnd in IN_BOUNDS:
                sl = slice(prev * GB, bnd * GB)
                prev = bnd
                for g in range(NTC):
                    nc.sync.dma_start(arenas[g][:, sl], mult_d[g][:, sl])

            out_bounds = sorted(
                set(range(32, HS - 32 + 1, 32)) | set(range(HS - 32, HS, 8))
            )
            prev_b = 0
            for k in range(HS):
                for g in range(NTC):
                    ps_t = psp.tile([P, GB], f32, tag=f"ps{g}")
                    nc.tensor.matmul(
                        ps_t[:],
                        w_t[:],
                        rings[g][:, k * GB : (k + 1) * GB],
                        start=True,
                        stop=True,
                    )
                    nc.vector.tensor_mul(
                        rings[g][:, (k + 1) * GB : (k + 2) * GB],
                        ps_t[:],
                        arenas[g][:, k * GB : (k + 1) * GB],
                    )
                if (k + 1) in out_bounds:
                    sl = slice(prev_b * GB, (k + 1) * GB)
                    prev_b = k + 1
                    for g in range(NTC):
                        nc.sync.dma_start(
                            hist_d[g][:, sl], rings[g][64 : 64 + T, sl]
                        )
            # tail: merged last-flush (covers through slot HS) + fin, issued
            # on two engine sequencers in parallel
            for g in range(NTC):
                eng = nc.scalar if g == 0 else nc.sync
                sl = slice(prev_b * GB, (HS + 1) * GB)
                eng.dma_start(hist_d[g][:, sl], rings[g][64 : 64 + T, sl])
                eng.dma_start(
                    fin_d[:, g * GB : (g + 1) * GB],
                    rings[g][0:T, HS * GB : (HS + 1) * GB],
                )
    _prune_ldweights(nc)
    nc.compile()
    return nc


def _prune_ldweights(nc):
    """All matmuls share one stationary: keep only the first InstLdweights
    (it carries the weight-DMA wait), mark matmuls non-self-loading."""
    first = True
    for blk in nc.m.functions[0].blocks:
        keep = []
        for inst in blk.instructions:
            tn = type(inst).__name__
            if tn == "InstMatmult":
                inst.ldweights = False
                keep.append(inst)
            elif tn == "InstLdweights":
                si = inst.sync_info
                has_sync = si is not None and (
                    len(si.on_wait) > 0 or len(si.on_update) > 0
                )
                if first or has_sync:
                    keep.append(inst)
                    first = False
            else:
                keep.append(inst)
        if len(keep) != len(blk.instructions):
            blk.instructions[:] = keep


def _bwd_pack(f64, lengths, cstart, cend):
    """Repacked bwd multiplier slots for chunk [cstart, cend).
    Returns (slot0 mult [B,T], device mults [B,HS,T], fold [B], jstar [B])."""
    top = np.minimum(lengths, cend)
    jstar = top - cstart                                 # [B] in [0, HS]
    slot = np.arange(NSLOT)[None, :]
    step_idx = top[:, None] - 1 - slot
    real = slot < jstar[:, None]
    step_clip = np.clip(step_idx, 0, S - 1)
    f_rep = f64[np.arange(B)[:, None], step_clip, :]
    g = (f_rep.mean(axis=-1) + FOLD_C) * real
    ef = np.exp(f_rep - g[..., None])
    ef = np.where(real[..., None], ef, 0.0)
    ef[np.arange(B), jstar, :] = 1.0
    return ef[:, 0, :], ef[:, 1:, :], g.sum(axis=1), jstar


def host_prep(feats, transitions, lengths):
    A = transitions.astype(np.float64)
    E = np.exp(A)
    q = np.exp(A[STOP_IDX, :])
    e_probe = np.ones(T)

    W = np.zeros((P, P), np.float64)
    W[0:T, 0:T] = E.T                 # fwd block: psum rows 0-47 = E @ state
    W[64 : 64 + T, 64 : 64 + T] = E   # bwd block: psum rows 64-111 = E^T @ w
    W_bf = W.astype(BF16)

    f64 = feats.astype(np.float64)
    gf = f64.mean(axis=-1) + FOLD_C                       # [B,S]
    ef_all = np.exp(f64 - gf[..., None])                  # [B,S,T]
    fold_f0 = gf[:, 0:B1].sum(axis=1)

    w0m_1, m_1, fold_b1, js1 = _bwd_pack(f64, lengths, B1, B2)
    w0m_2, m_2, fold_b2, js2 = _bwd_pack(f64, lengths, B2, S)
    ctop = lengths > B2                                   # c* == 2

    start1 = np.where(ctop[:, None], e_probe[None, :], q[None, :])
    w0_1 = w0m_1 * start1
    w0_2 = w0m_2 * q[None, :]

    inits, mults = [], []
    for c in range(NCORES):
        esl = slice(c * BSH, (c + 1) * BSH)
        init = np.zeros((P, NTC * GB), np.float64)
        ms = []
        for g in range(NTC):
            isl = slice(g * GB, (g + 1) * GB)
            if g == 0:
                init[START_IDX, isl] = 1.0                # alpha_0
                init[64 : 64 + T, isl] = w0_1[esl].T
                fm = ef_all[esl, 0:B1, :]                 # [GB,HS,T]
                bm = m_1[esl]
            else:
                init[0:T, isl] = e_probe[:, None]         # probe start
                init[64 : 64 + T, isl] = w0_2[esl].T
                fm = ef_all[esl, B1:B2, :]
                bm = m_2[esl]
            m = np.zeros((P, HS, GB), np.float64)
            m[0:T] = fm.transpose(2, 1, 0)
            m[64 : 64 + T] = bm.transpose(2, 1, 0)
            ms.append(np.ascontiguousarray(m.reshape(P, HS * GB)).astype(BF16))
        inits.append(init.astype(BF16))
        mults.append(ms)
    folds = dict(f0=fold_f0, b1=fold_b1, b2=fold_b2)
    return W_bf, inits, mults, folds, js1, js2, ctop, e_probe


def host_finish(hists, fins, folds, js1, js2, ctop, e_probe):
    """hists: [NCORES][NTC] of [T, NSLOT*GB]; fins: [NCORES][T, NTC*GB]."""
    out = np.empty((B,), np.float32)
    e_idx = np.arange(GB)
    for c in range(NCORES):
        bsl = slice(c * BSH, (c + 1) * BSH)
        fin = fins[c].astype(np.float64)
        a = fin[:, 0:GB]                                  # [T,GB] chunk0 fwd
        p1 = fin[:, GB : 2 * GB]                          # [T,GB] chunk1 probe
        h1 = hists[c][0].astype(np.float64).reshape(T, NSLOT, GB)
        h2 = hists[c][1].astype(np.float64).reshape(T, NSLOT, GB)
        v1 = h1[:, js1[bsl], e_idx]                       # [T,GB]
        v2 = h2[:, js2[bsl], e_idx]
        r1 = h1[:, HS, :]                                 # [T,GB] full chunk1
        low = np.log((a * v1).sum(axis=0)) + folds["f0"][bsl] + folds["b1"][bsl]
        high = (
            np.log((v2 * p1).sum(axis=0))
            - np.log((e_probe[:, None] * p1).sum(axis=0))
            + np.log((r1 * a).sum(axis=0))
            + folds["f0"][bsl]
            + folds["b1"][bsl]
            + folds["b2"][bsl]
        )
        out[bsl] = np.where(ctop[bsl], high, low).astype(np.float32)
    return out


def _run(feats, transitions, masks, trace=False):
    feats = np.asarray(feats)
    transitions = np.asarray(transitions)
    masks = np.asarray(masks)
    lengths = masks.sum(axis=1).astype(np.int64)

    W_bf, inits, mults, folds, js1, js2, ctop, e_probe = host_prep(
        feats, transitions, lengths
    )
    in_maps = [
        {
            "W": W_bf,
            "init": inits[c],
            **{f"mult{g}": mults[c][g] for g in range(NTC)},
        }
        for c in range(NCORES)
    ]
    nc = build_nc()
    bres = run_bass_kernel_spmd(
        nc, in_maps, core_ids=list(range(NCORES)), trace=trace
    )
    hists = [[r[f"hist{g}"] for g in range(NTC)] for r in bres.results]
    fins = [r["fin"] for r in bres.results]
    return host_finish(hists, fins, folds, js1, js2, ctop, e_probe), bres


def kernel(feats, transitions, masks):
    out, _ = _run(feats, transitions, masks, trace=False)
    return out
